# revision 19
# baseline (speedup 1.0000x reference)
"""BRD4KANModel Trainium2 kernel (v2: host-packed transposed bf16 weights).

Data-parallel over batch across 8 NeuronCores (512 rows each, weights
replicated). On-chip layout is feature-major (h^T: features on partitions,
batch on the free dim), so every layer's matmul output [out_feat, batch]
feeds the next layer directly.

v2 key change vs baseline: ALL weights are repacked on the HOST into the
exact transposed lhsT tile layout the PE needs ([in-on-partitions,
out-on-free], fused base+scaled-spline contraction, bf16), so the kernel
does ZERO on-chip transposes and zero on-chip weight scaling. Each KAN
layer's weight strip for one output tile is a single contiguous HBM region
streamed with large HWDGE DMAs.

B-spline bases use the truncated-power form: with r_m = relu(x - g_m),
the 6 cubic bases are lam^3 * (4th forward differences of r_m^3); lam^3
is folded into the packed spline weights on the host. The 4th differences
are computed with a fused scalar_tensor_tensor DAG (24 DVE ops per
feature tile instead of 30), and the r^2*r cube multiplies can be split
between DVE and GpSimd.

This walrus build accepts only ONE semaphore wait per instruction, while
Tile's scheduler attaches several; _split_waits() post-processes the BIR
JSON, hoisting excess waits onto NoOps inserted just before each
instruction on the same engine.
"""

import json
import os

import numpy as np
import ml_dtypes

import concourse.bass as bass
import concourse.mybir as mybir
import concourse.tile as tile
from concourse.masks import make_identity

F32 = mybir.dt.float32
BF16 = mybir.dt.bfloat16
AF = mybir.ActivationFunctionType
OP = mybir.AluOpType

N_CORES = 8
BATCH = 4096
B = BATCH // N_CORES  # 512 per core
D = 2048
WIDTHS = [2048, 2048, 1024]
DIMS = [D] + WIDTHS
COEFF = 6
GRID_SIZE = 3
SPLINE_ORDER = 3
H = 2.0 / GRID_SIZE
GRID = [m * H - 1.0 - SPLINE_ORDER * H for m in range(GRID_SIZE + 2 * SPLINE_ORDER + 1)]
LAM3 = float(1.0 / (6.0 * H ** 3))  # lam^3, folded into spline weights on host

BF = ml_dtypes.bfloat16

# how many of the 10 cube multiplies (r^2*r) go to GpSimd (rest on DVE)
N_GPS_MULT = int(os.environ.get("KGPS", "7"))


def _split_waits(bir_bytes: bytes, keep: int = 1) -> bytes:
    d = json.loads(bir_bytes)
    for f in d["functions"]:
        for bb in f["blocks"]:
            new_insts = []
            for inst in bb["instructions"]:
                si = inst.get("sync_info")
                waits = (si or {}).get("on_wait") or []
                if len(waits) > keep:
                    extra = waits[:-keep]
                    inst["sync_info"]["on_wait"] = waits[-keep:]
                    for ci in range(0, len(extra), keep):
                        new_insts.append({
                            "name": f"{inst['name']}-w{ci}",
                            "opcode": "NoOp",
                            "engine": inst["engine"],
                            "ins": [],
                            "outs": [],
                            "debug": inst.get("debug"),
                            "sync_info": {"on_update": [],
                                          "on_wait": extra[ci:ci + keep]},
                        })
                new_insts.append(inst)
            bb["instructions"] = new_insts
    return json.dumps(d).encode()


def _patch_json(nc):
    orig = nc.to_json_bytes

    def patched():
        return _split_waits(orig())

    nc.to_json_bytes = patched
    return nc


def build():
    nc = bass.Bass()
    # Host-packed tensors (see _pack_inputs):
    #  xT      [D, B]   bf16  per-core transposed input slice
    #  wm      [32, 128, 2048]        bf16  mult_w lhsT strips
    #  wk{l}   [OT, 128, IT*7*128]    bf16  fused base+spline lhsT strips
    #  mbp     [128, 32] f32  mult_b per-partition
    #  hwp     [128, 16] bf16 reg_w/aux_w lhsT
    #  hbp     [2, 1]    f32  reg_b/aux_b
    xT = nc.dram_tensor("xT", [D, B], BF16, kind="ExternalInput")
    wm = nc.dram_tensor("wm", [32, 128, 2048], BF16, kind="ExternalInput")
    wk = []
    for l in range(3):
        ot, it = DIMS[l + 1] // 128, DIMS[l] // 128
        wk.append(nc.dram_tensor(f"wk{l}", [ot, 128, it * 7 * 128], BF16,
                                 kind="ExternalInput"))
    mbp = nc.dram_tensor("mbp", [128, 32], F32, kind="ExternalInput")
    hwp = nc.dram_tensor("hwp", [128, 16], BF16, kind="ExternalInput")
    hbp = nc.dram_tensor("hbp", [2, 1], F32, kind="ExternalInput")
    out = nc.dram_tensor("out", [2, B], F32, kind="ExternalOutput")

    KCH = 28  # k-tiles per weight-DMA chunk (28*128*128*2B = 0.92MB)

    with tile.TileContext(nc) as tc:
        with tc.tile_pool(name="consts", bufs=1) as consts, \
             tc.tile_pool(name="hp", bufs=17) as hp, \
             tc.tile_pool(name="sv", bufs=3) as svp, \
             tc.tile_pool(name="rhs", bufs=17) as rhsp, \
             tc.tile_pool(name="bases", bufs=96) as basesp, \
             tc.tile_pool(name="zp", bufs=11) as zp, \
             tc.tile_pool(name="up", bufs=6) as upool, \
             tc.tile_pool(name="wload", bufs=2) as wload, \
             tc.tile_pool(name="h0", bufs=15) as h0p, \
             tc.tile_pool(name="h2", bufs=2) as h2p, \
             tc.tile_pool(name="psA", bufs=8, space="PSUM") as psA:

            mb_sb = consts.tile([128, 32], F32, tag="mb")
            nc.sync.dma_start(mb_sb, mbp[:])
            hw_sb = consts.tile([128, 16], BF16, tag="hw")
            nc.sync.dma_start(hw_sb, hwp[:])
            hb_sb = consts.tile([2, 1], F32, tag="hb")
            nc.sync.dma_start(hb_sb, hbp[:])
            grid_sb = consts.tile([128, 10], F32, tag="grid")
            for m in range(10):
                nc.vector.memset(grid_sb[:, m:m + 1], float(-GRID[m]))
            ident = consts.tile([128, 128], BF16, tag="ident")
            make_identity(nc, ident)

            # ---- x^T load: 16 rhs tiles [128, B] bf16, features on parts ----
            IT0 = D // 128  # 16
            xb = []
            for i in range(IT0):
                t = rhsp.tile([128, B], BF16, tag="rhs", name=f"xb{i}")
                nc.sync.dma_start(t, xT[i * 128:(i + 1) * 128, :])
                xb.append(t)

            silu_t = {}
            bas_t = {}

            def phase_a(l, i, h_in):
                """silu + spline bases for feature tile i of layer l's input.

                GpSimd handles the cube-multiplies for the HIGH knots
                (m >= 10-N_GPS_MULT) so the DVE difference DAG (whose first
                ops touch low-m z's) never waits on the slower GpSimd."""
                st = rhsp.tile([128, B], BF16, tag="rhs", name=f"silu{l}_{i}")
                nc.scalar.activation(st, h_in[i], AF.Silu)
                silu_t[(l, i)] = st
                # r_m^3 for the 10 shifted relus
                z = []
                z2 = [zp.tile([128, B], F32, tag="z", name=f"z2t{q}")
                      for q in range(2)]
                for m in range(10):
                    zm = zp.tile([128, B], F32, tag="z", name=f"z{m}")
                    nc.scalar.activation(zm, h_in[i], AF.Relu,
                                         bias=grid_sb[:, m:m + 1])
                    nc.scalar.square(z2[m % 2], zm)
                    if m >= 10 - N_GPS_MULT:
                        nc.gpsimd.tensor_tensor(zm, z2[m % 2], zm, OP.mult)
                    else:
                        nc.vector.tensor_tensor(zm, z2[m % 2], zm, OP.mult)
                    z.append(zm)
                # 4th differences via fused DAG:
                #   u_c = z[c+1] + z[c+3]
                #   s_c = z[c] + z[c+4]         (in place into z[c])
                #   t_c = u_c * (-4) + s_c      (in place into u_c)
                #   b_c = z[c+2] * 6 + t_c      -> bf16 bases tile
                # DVE-only-dependency ops (low m) are emitted first so the
                # DVE stream overlaps the GpSimd high-m cubes.
                bt6 = [basesp.tile([128, B], BF16, tag="bases",
                                   name=f"bas{l}_{i}_{c}") for c in range(COEFF)]
                bas_t[(l, i)] = bt6
                u = [upool.tile([128, B], F32, tag="u", name=f"u{c}")
                     for c in range(COEFF)]
                nc.vector.tensor_tensor(u[0], z[1], z[3], OP.add)
                nc.vector.tensor_tensor(u[1], z[2], z[4], OP.add)
                nc.vector.tensor_tensor(z[0], z[0], z[4], OP.add)
                for c in range(2, COEFF):
                    nc.vector.tensor_tensor(u[c], z[c + 1], z[c + 3], OP.add)
                nc.vector.tensor_tensor(z[1], z[1], z[5], OP.add)
                for c in range(COEFF):
                    nc.vector.scalar_tensor_tensor(u[c], u[c], -4.0, z[c],
                                                   OP.mult, OP.add)
                    nc.vector.scalar_tensor_tensor(bt6[c], z[c + 2], 6.0, u[c],
                                                   OP.mult, OP.add)
                    if c + 2 < COEFF:
                        nc.vector.tensor_tensor(z[c + 2], z[c + 2], z[c + 6],
                                                OP.add)

            def kan_matmul_group(l, o, h0, i_lo, i_hi):
                """one PSUM accumulation group: output tile o of layer l,
                contracting feature tiles [i_lo, i_hi).

                First half (h0 None): evacuate the partial sum to a bf16
                tile and return it. Second half: re-inject the bf16 partial
                via an identity matmul, then return the LIVE PSUM tile —
                for l<2 phase_a reads h straight out of PSUM (ACT ops only),
                for l==2 the caller evacuates to bf16 for the heads."""
                acc = psA.tile([128, B], F32, tag="acc")
                n_k = (i_hi - i_lo) * 7 + (0 if h0 is None else 1)
                k = 0
                if h0 is not None:
                    nc.tensor.matmul(acc, ident, h0, start=True, stop=False)
                    k = 1
                for c0 in range(i_lo * 7, i_hi * 7, KCH):
                    cw = min(KCH, i_hi * 7 - c0)
                    wt = wload.tile([128, KCH * 128], BF16, tag="w")
                    nc.sync.dma_start(
                        wt[:, :cw * 128],
                        wk[l][o, :, c0 * 128:(c0 + cw) * 128])
                    for kk in range(cw):
                        it, comp = divmod(c0 + kk, 7)
                        rhs_t = (silu_t[(l, it)] if comp == 0
                                 else bas_t[(l, it)][comp - 1])
                        nc.tensor.matmul(acc, wt[:, kk * 128:(kk + 1) * 128],
                                         rhs_t, start=(k == 0),
                                         stop=(k == n_k - 1))
                        k += 1
                if h0 is None:
                    ht = h0p.tile([128, B], BF16, tag="h0", name=f"h0_{l}_{o}")
                    nc.scalar.copy(ht, acc)
                    return ht
                return acc

            # ---- multiplicative layer (evacs avoid DVE: ACT + GpSimd) ----
            h_tiles = []
            for j in range(IT0):
                evac = {}
                for o in (j, j + 16):  # gate tile then val tile
                    acc = psA.tile([128, B], F32, tag="acc")
                    wt = wload.tile([128, KCH * 128], BF16, tag="w")
                    nc.sync.dma_start(wt[:, :2048], wm[o])
                    for i in range(IT0):
                        nc.tensor.matmul(acc, wt[:, i * 128:(i + 1) * 128],
                                         xb[i], start=(i == 0),
                                         stop=(i == IT0 - 1))
                    evac[o] = acc
                sig = svp.tile([128, B], F32, tag="sv")
                nc.scalar.activation(sig, evac[j], AF.Sigmoid,
                                     bias=mb_sb[:, j:j + 1])
                val = svp.tile([128, B], F32, tag="sv")
                nc.scalar.activation(val, evac[j + 16], AF.Identity,
                                     bias=mb_sb[:, 16 + j:17 + j])
                ht = hp.tile([128, B], BF16, tag="h")
                nc.gpsimd.tensor_tensor(ht, sig, val, OP.mult)
                h_tiles.append(ht)
            # phase_a(0) emitted AFTER the whole mult layer so its DVE work
            # queues behind (not ahead of) the mult PSUM evacuations.
            for j in range(IT0):
                phase_a(0, j, h_tiles)

            # ---- KAN layers: 2-sweep k-split pipeline ----
            # B-half0(l) overlaps phase_a of (l, i-half1)... emission order
            # mirrors the baseline: A(l,half1) already queued; B0(l) then
            # B1(l); phase_a(l+1, i) emitted right after B1(l) group i.
            cur_h = h_tiles
            head_acc = None
            for l in range(3):
                fi, fo = DIMS[l], DIMS[l + 1]
                IT, OT = fi // 128, fo // 128
                h0s = []
                for o in range(OT):
                    h0s.append(kan_matmul_group(l, o, None, 0, IT // 2))
                new_h = []
                for o in range(OT):
                    acc = kan_matmul_group(l, o, h0s[o], IT // 2, IT)
                    if l < 2:
                        # h(l+1)[o] stays in PSUM; phase_a reads it directly
                        # (ACT-only consumers), freeing the bank afterwards.
                        new_h.append(acc)
                        phase_a(l + 1, o, new_h)
                    else:
                        # evacuate to bf16 and fold the head matmul in right
                        # away so only 2 h2 buffers are ever live.
                        ot_t = h2p.tile([128, B], BF16, tag="h2",
                                        name=f"h2_{o}")
                        nc.scalar.copy(ot_t, acc)
                        if head_acc is None:
                            head_acc = psA.tile([128, B], F32, tag="acc",
                                                name="head_acc")
                        nc.tensor.matmul(head_acc[0:2, :],
                                         hw_sb[:, o * 2:o * 2 + 2], ot_t,
                                         start=(o == 0), stop=(o == OT - 1))
                        new_h.append(ot_t)
                cur_h = new_h

            # ---- heads bias + store ----
            res = consts.tile([2, B], F32, tag="res")
            nc.vector.tensor_scalar(res, head_acc[0:2, :], hb_sb[:, 0:1],
                                    None, OP.add)
            nc.sync.dma_start(out[:], res)

    return _patch_json(nc)


def _pack_inputs(inputs):
    """Host-side repack: transpose/fuse/scale all weights into lhsT tile
    layout, cast to bf16. Returns dict of packed shared arrays."""
    f32 = np.float32
    packed = {}
    # mult_w [4096, 2048] -> [32 ot, 128 p, 16 it * 128 m]
    mw = np.asarray(inputs["mult_w"], f32)
    packed["wm"] = np.ascontiguousarray(
        mw.reshape(32, 128, 16, 128).transpose(0, 3, 2, 1), dtype=BF
    ).reshape(32, 128, 2048)
    for l in range(3):
        fi, fo = DIMS[l], DIMS[l + 1]
        ot_n, it_n = fo // 128, fi // 128
        W = np.empty((fo, fi, 7), f32)
        W[:, :, 0] = np.asarray(inputs[f"base_w{l}"], f32)
        sc = np.asarray(inputs[f"scaler{l}"], f32) * LAM3
        W[:, :, 1:] = np.asarray(inputs[f"spline_w{l}"], f32) * sc[:, :, None]
        A = W.reshape(ot_n, 128, it_n, 128, 7).transpose(0, 3, 2, 4, 1)
        packed[f"wk{l}"] = np.ascontiguousarray(A, dtype=BF).reshape(
            ot_n, 128, it_n * 7 * 128)
    packed["mbp"] = np.ascontiguousarray(
        np.asarray(inputs["mult_b"], f32).reshape(32, 128).T)
    hw = np.stack([np.asarray(inputs["reg_w"], f32)[0],
                   np.asarray(inputs["aux_w"], f32)[0]], axis=1)  # (1024, 2)
    packed["hwp"] = np.ascontiguousarray(
        hw.reshape(8, 128, 2).transpose(1, 0, 2), dtype=BF).reshape(128, 16)
    packed["hbp"] = np.array([[float(np.asarray(inputs["reg_b"])[0])],
                              [float(np.asarray(inputs["aux_b"])[0])]], f32)
    return packed


_NC = None


def kernel(**inputs):
    global _NC
    from concourse.bass_utils import run_bass_kernel_spmd

    if _NC is None:
        _NC = build()
    shared = _pack_inputs(inputs)
    x_full = np.asarray(inputs["x"], np.float32)
    per_core = []
    for c in range(N_CORES):
        m = dict(shared)
        m["xT"] = np.ascontiguousarray(x_full[c * B:(c + 1) * B].T, dtype=BF)
        per_core.append(m)
    res = run_bass_kernel_spmd(_NC, per_core, core_ids=list(range(N_CORES)))
    reg = np.concatenate([res.results[c]["out"][0] for c in range(N_CORES)])
    aux = np.concatenate([res.results[c]["out"][1] for c in range(N_CORES)])
    kernel.last_results = res
    return reg, aux
